# revision 9
# baseline (speedup 1.0000x reference)
"""Trainium2 Bass kernel for nn_AttnEncoder (2-branch 2-layer GAT + attn pred).

Self-contained: hardcodes problem shapes; preprocesses edges on host, compiles
one SPMD Bass program, runs on 8 NeuronCores via PJRT, returns full output.

Sharding: nodes (and their dst-sorted incoming edges) are partitioned across
8 cores; GAT weights replicated; features all-gathered between layers.

Per GAT conv (graph g, layer l):
  - node-table rows [s | h(128) | wslot | pad] (bf16, 512B rows) built on the
    PE (h' = x @ [Wa_src | W | Wa_dst]) and written to HBM; a local d-table
    [n_loc, 128] holds d broadcast per row.
  - edges (sorted by dst, grouped per 128-dst tile, bucketed by src range so
    gather indices fit int16) fetch rows edge-major via SWDGE dma_gather.
  - per-edge weights w = exp(leaky_relu(s_src + d_dst)) are computed batched
    and applied with one broadcast-AP tensor_tensor per dst tile; a streamed
    static fp8 one-hot matmul scatter-accumulates [128 dst, 128 feat + denom]
    into PSUM; epilogue divides by the denominator and adds bias (+relu L1).
Between layers: AllGather of feature-major local blocks (global x kept
block-feat-major [8, 128, n_loc] so no device transposes are needed).
"""
import sys
import numpy as np

sys.path.insert(0, "/opt/trn_rl_repo")

import jax
from jax.sharding import Mesh, PartitionSpec
from jax.experimental.shard_map import shard_map

import concourse.bass as bass
import concourse.bacc as bacc
import concourse.mybir as mybir
from concourse import tile
from concourse.library_config import mlp
from concourse.bass2jax import _bass_exec_p, install_neuronx_cc_hook, partition_id_tensor

F32 = mybir.dt.float32
BF16 = mybir.dt.bfloat16
FP8 = mybir.dt.float8e4
I16 = mybir.dt.int16
AF = mybir.ActivationFunctionType
ALU = mybir.AluOpType

NEG_SLOPE = 0.2

REAL_CFG = dict(N=50000, NCORES=8, PER_CORE=6272, DDEG=32)


def _derive(cfg):
    cfg = dict(cfg)
    cfg["NPAD"] = cfg["PER_CORE"] * cfg["NCORES"]
    cfg["NT"] = cfg["PER_CORE"] // 128
    cfg["BUCKET"] = cfg["NPAD"] // 2
    assert cfg["BUCKET"] <= 32767
    return cfg


# ---------------------------------------------------------------- host prep

def _wrap16(idx: np.ndarray) -> np.ndarray:
    """int idx [n] (n%16==0) -> [128, n//16] int16 wrapped in 16 partitions,
    replicated 8x vertically (one copy per Q7 core)."""
    n = idx.shape[0]
    w = idx.reshape(n // 16, 16).T.astype(np.int16)
    return np.tile(w, (8, 1))


def prep_graph(edge_index: np.ndarray, cfg):
    """Returns (per_core list of dicts {idx, didx, oh}, sched list[(P0,P1)])."""
    NPAD, PER_CORE, NT, BUCKET, NC = (cfg["NPAD"], cfg["PER_CORE"], cfg["NT"],
                                      cfg["BUCKET"], cfg["NCORES"])
    src = edge_index[0].astype(np.int64)
    dst = edge_index[1].astype(np.int64)
    loops = np.arange(NPAD, dtype=np.int64)
    src = np.concatenate([src, loops])
    dst = np.concatenate([dst, loops])

    core = dst // PER_CORE
    dst_local = dst - core * PER_CORE
    tile_id = dst_local >> 7
    bucket = src // BUCKET

    key = (core * NT + tile_id) * 2 + bucket
    cnt = np.bincount(key, minlength=NC * NT * 2).reshape(NC, NT, 2)
    P = np.maximum.reduce(-(-cnt // 128), axis=0)          # [NT, 2]
    sched = [(int(P[t, 0]), int(P[t, 1])) for t in range(NT)]

    order = np.argsort(key, kind="stable")
    src_s, dstl_s, key_s = src[order], dst_local[order], key[order]
    seg_lo = np.searchsorted(key_s, np.arange(NC * NT * 2))
    seg_hi = np.searchsorted(key_s, np.arange(NC * NT * 2), side="right")

    fp8np = mybir.dt.np(FP8)
    per_core = []
    for c in range(NC):
        idx_parts, didx_parts, oh_parts = [], [], []
        for t in range(NT):
            d_all = []
            for b in range(2):
                p = P[t, b]
                if p == 0:
                    continue
                k = (c * NT + t) * 2 + b
                lo, hi = seg_lo[k], seg_hi[k]
                n = hi - lo
                npd = p * 128
                e_src = np.zeros(npd, dtype=np.int64)
                e_src[:n] = src_s[lo:hi] - b * BUCKET
                e_dl = np.full(npd, -1, dtype=np.int64)
                e_dl[:n] = dstl_s[lo:hi]
                idx_parts.append(_wrap16(e_src))
                d_all.append(e_dl)
            e_dl = np.concatenate(d_all)
            nch = e_dl.shape[0] // 128
            oh = np.zeros((128, nch * 128), dtype=np.float32)
            ohT = np.zeros((128, nch * 128), dtype=np.float32)
            j = np.arange(e_dl.shape[0])
            valid = e_dl >= 0
            jv = j[valid]
            col = (jv // 128) * 128 + (e_dl[valid] & 127)
            oh[jv % 128, col] = 1.0
            ohT[e_dl[valid] & 127, (jv // 128) * 128 + (jv % 128)] = 1.0
            oh_parts.append(oh)
            didx_parts.append(ohT)
        per_core.append({
            "idx": np.concatenate(idx_parts, axis=1),
            "ohT": np.concatenate(didx_parts, axis=1).astype(fp8np),
            "oh": np.concatenate(oh_parts, axis=1).astype(fp8np),
        })
    return per_core, sched


# ---------------------------------------------------------------- program

def build_program(cfg, sched_o, sched_s, sim_safe=False):
    NPAD, PER_CORE, NT, BUCKET, NC, DDEG = (cfg["NPAD"], cfg["PER_CORE"], cfg["NT"],
                                            cfg["BUCKET"], cfg["NCORES"], cfg["DDEG"])
    D = 128
    sched = {"o": sched_o, "s": sched_s}
    Wi = {g: sum(p0 + p1 for p0, p1 in sched[g]) * 8 for g in ("o", "s")}
    Etot = {g: sum(p0 + p1 for p0, p1 in sched[g]) * 128 for g in ("o", "s")}

    nc = bacc.Bacc("TRN2")

    xTg = nc.dram_tensor("xTg", [NC, 128, PER_CORE], BF16, kind="ExternalInput")
    x_locT = nc.dram_tensor("x_locT", [128, PER_CORE], BF16, kind="ExternalInput")
    degT_d = nc.dram_tensor("degT", [DDEG, PER_CORE], BF16, kind="ExternalInput")
    ident_d = nc.dram_tensor("ident", [128, 128], BF16, kind="ExternalInput")
    Wp_d, bias_d = {}, {}
    for g in ("o", "s"):
        for l in (1, 2):
            Wp_d[g, l] = nc.dram_tensor(f"Wp_{g}{l}", [128, 130], BF16, kind="ExternalInput")
            bias_d[g, l] = nc.dram_tensor(f"bias_{g}{l}", [128, 128], F32, kind="ExternalInput")
    Wpred_d = {k: nc.dram_tensor(f"Wpred_{k}", [128 if k != "c" else DDEG, 2], BF16,
                                 kind="ExternalInput") for k in ("a", "b", "c")}
    bpred_d = nc.dram_tensor("bpred_diff", [128, 1], F32, kind="ExternalInput")
    idx_d = {g: nc.dram_tensor(f"idx_{g}", [128, Wi[g]], I16, kind="ExternalInput")
             for g in ("o", "s")}
    oh_d = {g: nc.dram_tensor(f"oh_{g}", [128, Etot[g]], FP8, kind="ExternalInput")
            for g in ("o", "s")}
    ohT_d = {g: nc.dram_tensor(f"ohT_{g}", [128, Etot[g]], FP8, kind="ExternalInput")
             for g in ("o", "s")}

    pred_out = nc.dram_tensor("pred_out", [PER_CORE, D], F32, kind="ExternalOutput")

    table = [nc.dram_tensor(f"table{i}", [NPAD, 256], BF16) for i in range(2)]
    ag_in = {g: nc.dram_tensor(f"ag_in_{g}", [128, PER_CORE], BF16) for g in ("o", "s")}
    ag_out = {g: nc.dram_tensor(f"ag_out_{g}", [NC, 128, PER_CORE], BF16,
                                addr_space="Shared") for g in ("o", "s")}

    with tile.TileContext(nc) as tc:
        nc.gpsimd.load_library(mlp)
        with (
            tc.tile_pool(name="pers", bufs=1) as pers_p,
            tc.tile_pool(name="xblk", bufs=2) as xblk_p,
            tc.tile_pool(name="stage", bufs=4) as stage_p,
            tc.tile_pool(name="idxp", bufs=1) as idx_p,
            tc.tile_pool(name="gat", bufs=2) as gat_p,
            tc.tile_pool(name="ohp", bufs=3) as oh_p,
            tc.tile_pool(name="sc", bufs=6) as sc_p,
            tc.tile_pool(name="ep", bufs=4) as ep_p,
            tc.tile_pool(name="x1out", bufs=3) as x1out_p,
            tc.tile_pool(name="prp", bufs=4) as pr_p,
            tc.tile_pool(name="psT", bufs=2, space="PSUM") as psT_p,
            tc.tile_pool(name="psA", bufs=2, space="PSUM") as psA_p,
            tc.tile_pool(name="psR", bufs=2, space="PSUM") as psR_p,
            tc.tile_pool(name="psD", bufs=1, space="PSUM") as psD_p,
            tc.tile_pool(name="psL", bufs=1, space="PSUM") as psL_p,
        ):
            ident = pers_p.tile([128, 128], BF16, tag="ident")
            nc.sync.dma_start(out=ident[:], in_=ident_d[:])
            x_locT_sb = pers_p.tile([128, PER_CORE], BF16, tag="xlocT")
            nc.sync.dma_start(out=x_locT_sb[:], in_=x_locT[:])
            degT_sb = pers_p.tile([DDEG, PER_CORE], BF16, tag="degT")
            nc.sync.dma_start(out=degT_sb[:], in_=degT_d[:])
            bpred_sb = pers_p.tile([128, 1], F32, tag="bpred")
            nc.sync.dma_start(out=bpred_sb[:], in_=bpred_d[:])

            wp_sb, bias_sb = {}, {}
            for g in ("o", "s"):
                for l in (1, 2):
                    w_ = pers_p.tile([128, 130], BF16, tag=f"wp{g}{l}", name=f"wp{g}{l}")
                    nc.sync.dma_start(out=w_[:], in_=Wp_d[g, l][:])
                    wp_sb[g, l] = w_
                    b_ = pers_p.tile([128, 128], F32, tag=f"bb{g}{l}", name=f"bb{g}{l}")
                    nc.sync.dma_start(out=b_[:], in_=bias_d[g, l][:])
                    bias_sb[g, l] = b_
            wpred_sb = {}
            for k in ("a", "b", "c"):
                t_ = pers_p.tile([128 if k != "c" else DDEG, 2], BF16, tag=f"wpred{k}", name=f"wpred{k}")
                nc.sync.dma_start(out=t_[:], in_=Wpred_d[k][:])
                wpred_sb[k] = t_

            x1_locT = {g: pers_p.tile([128, NT * 128], BF16, tag=f"x1T{g}", name=f"x1T{g}")
                       for g in ("o", "s")}
            d_loc = [pers_p.tile([128, NT], BF16, tag=f"dloc{i}", name=f"dloc{i}") for i in range(2)]
            x2_nm = {g: pers_p.tile([128, NT * 128], BF16, tag=f"x2nm{g}", name=f"x2nm{g}")
                     for g in ("o", "s")}

            def build_table(ti, xsrc_dram, g, l, loc_lhsT):
                """Node table (h' rows) for conv (g,l) -> table[ti]; local
                d-table -> dtab[ti]. xsrc_dram: [NC, 128, PER_CORE] bf16."""
                wt = wp_sb[g, l]
                for blk in range(NC):
                    xb = xblk_p.tile([128, PER_CORE], BF16, tag="xb")
                    nc.sync.dma_start(out=xb[:], in_=xsrc_dram[blk])
                    t = 0
                    while t < NT:
                        nb = min(3, NT - t)
                        ps = psT_p.tile([128, 3 * 130], F32, tag="pT")
                        for u in range(nb):
                            nc.tensor.matmul(
                                ps[:, u * 130:(u + 1) * 130],
                                lhsT=xb[:, (t + u) * 128:(t + u + 1) * 128],
                                rhs=wt[:], start=True, stop=True)
                        st = stage_p.tile([128, 3, 256], BF16, tag="st")
                        if sim_safe:
                            nc.vector.memset(st[:, :, 129:256], 0.0)
                        if (t // 3) % 2 == 0:
                            nc.vector.tensor_copy(
                                out=st[:, 0:nb, 0:129],
                                in_=ps[:].rearrange("p (u q) -> p u q", q=130)[:, 0:nb, 0:129])
                        else:
                            nc.scalar.copy(
                                out=st[:, 0:nb, 0:129],
                                in_=ps[:].rearrange("p (u q) -> p u q", q=130)[:, 0:nb, 0:129])
                        nc.sync.dma_start(
                            out=table[ti][(blk * NT + t) * 128:(blk * NT + t + nb) * 128, :]
                                .rearrange("(u p) q -> p u q", p=128),
                            in_=st[:, 0:nb, :])
                        t += nb
                t = 0
                while t < NT:
                    nb = min(3, NT - t)
                    ps = psT_p.tile([128, 3], F32, tag="pT")
                    for u in range(nb):
                        nc.tensor.matmul(
                            ps[:, u:u + 1],
                            lhsT=loc_lhsT[:, (t + u) * 128:(t + u + 1) * 128],
                            rhs=wt[:, 129:130], start=True, stop=True)
                    nc.vector.tensor_copy(out=d_loc[ti][:, t:t + nb], in_=ps[:, 0:nb])
                    t += nb

            def conv(g, l, ti, relu, out_nm_of, out_T_of):
                """One GAT conv; out_nm_of(t)/out_T_of(t) -> dest APs [128,128]."""
                S = sched[g]
                idx_sb = idx_p.tile([128, max(Wi["o"], Wi["s"])], I16, tag="idx")
                nc.sync.dma_start(out=idx_sb[:, 0:Wi[g]], in_=idx_d[g][:])

                icol = ohcol = 0
                for t in range(NT):
                    P0, P1 = S[t]
                    PT = P0 + P1
                    gt = gat_p.tile([128, PT, 256], BF16, tag="g")
                    for b, off, Pb in ((0, 0, P0), (1, P0, P1)):
                        if Pb == 0:
                            continue
                        nc.gpsimd.dma_gather(
                            out_ap=gt[:, off:off + Pb, :],
                            in_ap=table[ti][b * BUCKET:(b + 1) * BUCKET, :],
                            idxs_ap=idx_sb[:, icol:icol + Pb * 8],
                            num_idxs=Pb * 128, num_idxs_reg=Pb * 128,
                            elem_size=256, single_packet=False)
                        icol += Pb * 8
                    oht = oh_p.tile([128, PT * 128], FP8, tag="oh")
                    nc.sync.dma_start(out=oht[:], in_=oh_d[g][:, ohcol:ohcol + PT * 128])
                    ohtT = oh_p.tile([128, PT * 128], FP8, tag="ohT")
                    nc.sync.dma_start(out=ohtT[:], in_=ohT_d[g][:, ohcol:ohcol + PT * 128])
                    ohcol += PT * 128

                    pd = psD_p.tile([128, PT], F32, tag="pd")
                    for k in range(PT):
                        nc.tensor.matmul(
                            pd[:, k:k + 1], lhsT=ohtT[:, k * 128:(k + 1) * 128],
                            rhs=d_loc[ti][:, t:t + 1],
                            start=True, stop=True)
                    t0 = sc_p.tile([128, PT], F32, tag="t0")
                    nc.vector.tensor_tensor(out=t0[:], in0=gt[:, :, 0], in1=pd[:],
                                            op=ALU.add)
                    t1 = sc_p.tile([128, PT], F32, tag="t1")
                    nc.vector.scalar_tensor_tensor(
                        out=t1[:], in0=t0[:], scalar=NEG_SLOPE, in1=t0[:],
                        op0=ALU.mult, op1=ALU.max)
                    wcol = sc_p.tile([128, PT], BF16, tag="w")
                    nc.scalar.activation(wcol[:], t1[:], AF.Exp)
                    nc.vector.memset(gt[:, :, 129], 1.0)
                    nc.vector.tensor_tensor(
                        out=gt[:, :, 1:130], in0=gt[:, :, 1:130],
                        in1=wcol[:].unsqueeze(2).to_broadcast([128, PT, 129]),
                        op=ALU.mult)

                    pa = psA_p.tile([128, 129], F32, tag="pa")
                    for k in range(PT):
                        nc.tensor.matmul(
                            pa[:], lhsT=oht[:, k * 128:(k + 1) * 128],
                            rhs=gt[:, k, 1:130],
                            start=(k == 0), stop=(k == PT - 1))

                    rcol = ep_p.tile([128, 1], F32, tag="r")
                    nc.vector.reciprocal(rcol[:], pa[:, 128:129])
                    out_nm = out_nm_of(t)
                    if relu:
                        of = ep_p.tile([128, 128], F32, tag="of")
                        nc.vector.scalar_tensor_tensor(
                            out=of[:], in0=pa[:, 0:128], scalar=rcol[:],
                            in1=bias_sb[g, l][:], op0=ALU.mult, op1=ALU.add)
                        nc.scalar.activation(out_nm, of[:], AF.Relu)
                    else:
                        nc.vector.scalar_tensor_tensor(
                            out=out_nm, in0=pa[:, 0:128], scalar=rcol[:],
                            in1=bias_sb[g, l][:], op0=ALU.mult, op1=ALU.add)
                    if out_T_of is not None:
                        pt_ = psR_p.tile([128, 128], BF16, tag="ptr")
                        nc.tensor.transpose(out=pt_[:], in_=out_nm, identity=ident[:])
                        nc.scalar.copy(out=out_T_of(t), in_=pt_[:])

            # ---------------- layer 1 (both branches) + allgather
            def run_branch_l1(g, ti):
                build_table(ti, xTg, g, 1, x_locT_sb)

                def nm_of(t):
                    tl = x1out_p.tile([128, 128], BF16, tag="x1nm")
                    return tl[:]

                conv(g, 1, ti, relu=True, out_nm_of=nm_of,
                     out_T_of=lambda t, g=g: x1_locT[g][:, t * 128:(t + 1) * 128])
                nc.sync.dma_start(out=ag_in[g][:], in_=x1_locT[g][:])
                nc.gpsimd.collective_compute(
                    "AllGather", ALU.bypass,
                    replica_groups=[list(range(NC))],
                    ins=[ag_in[g][:]], outs=[ag_out[g][:]])

            run_branch_l1("o", 0)
            run_branch_l1("s", 1)

            # ---------------- layer 2
            for gi, g in enumerate(("o", "s")):
                build_table(gi, ag_out[g], g, 2, x1_locT[g])
                conv(g, 2, gi, relu=False,
                     out_nm_of=lambda t, g=g: x2_nm[g][:, t * 128:(t + 1) * 128],
                     out_T_of=None)

            # ---------------- pred
            for t in range(NT):
                lg = psL_p.tile([128, 2], F32, tag="lg")
                first = True
                for g, wk in (("o", "a"), ("s", "b")):
                    ptr_ = psR_p.tile([128, 128], BF16, tag="ptr")
                    nc.tensor.transpose(out=ptr_[:], in_=x2_nm[g][:, t * 128:(t + 1) * 128],
                                        identity=ident[:])
                    x2T = pr_p.tile([128, 128], BF16, tag="x2T")
                    nc.scalar.copy(out=x2T[:], in_=ptr_[:])
                    nc.tensor.matmul(lg[:], lhsT=x2T[:], rhs=wpred_sb[wk][:],
                                     start=first, stop=False)
                    first = False
                nc.tensor.matmul(lg[:], lhsT=degT_sb[:, t * 128:(t + 1) * 128],
                                 rhs=wpred_sb["c"][:], start=False, stop=True)
                lgs = pr_p.tile([128, 2], F32, tag="lgs")
                nc.vector.tensor_copy(out=lgs[:], in_=lg[:])
                diff = pr_p.tile([128, 1], F32, tag="diff")
                nc.vector.tensor_tensor(out=diff[:], in0=lgs[:, 0:1], in1=lgs[:, 1:2],
                                        op=ALU.subtract)
                attn = pr_p.tile([128, 1], F32, tag="attn")
                nc.scalar.activation(attn[:], diff[:], AF.Sigmoid, bias=bpred_sb[:])
                delta = pr_p.tile([128, 128], BF16, tag="delta")
                nc.vector.tensor_tensor(
                    out=delta[:], in0=x2_nm["o"][:, t * 128:(t + 1) * 128],
                    in1=x2_nm["s"][:, t * 128:(t + 1) * 128], op=ALU.subtract)
                po = pr_p.tile([128, 128], F32, tag="po")
                nc.vector.scalar_tensor_tensor(
                    out=po[:], in0=delta[:], scalar=attn[:],
                    in1=x2_nm["s"][:, t * 128:(t + 1) * 128],
                    op0=ALU.mult, op1=ALU.add)
                nc.sync.dma_start(out=pred_out[t * 128:(t + 1) * 128, :], in_=po[:])

    nc.compile()
    return nc


# ---------------------------------------------------------------- runner

class SpmdRunner:
    def __init__(self, nc, n_cores):
        install_neuronx_cc_hook()
        self.nc = nc
        self.n_cores = n_cores
        in_names, out_names, out_avals, zero_outs = [], [], [], []
        pname = nc.partition_id_tensor.name if nc.partition_id_tensor else None
        for alloc in nc.m.functions[0].allocations:
            if not isinstance(alloc, mybir.MemoryLocationSet):
                continue
            name = alloc.memorylocations[0].name
            if alloc.kind == "ExternalInput":
                if name != pname:
                    in_names.append(name)
            elif alloc.kind == "ExternalOutput":
                out_names.append(name)
                shape = tuple(alloc.tensor_shape)
                dtype = mybir.dt.np(alloc.dtype)
                out_avals.append(jax.core.ShapedArray(shape, dtype))
                zero_outs.append(np.zeros(shape, dtype))
        self.dbg_name = nc.dbg_addr.name if nc.dbg_addr is not None else None
        self.in_names = [n for n in in_names if n != self.dbg_name]
        self.out_names, self.out_avals, self.zero_outs = out_names, out_avals, zero_outs
        all_in = list(self.in_names)
        if self.dbg_name is not None:
            all_in.append(self.dbg_name)
        self.n_params = len(all_in)
        all_in.extend(out_names)
        if pname is not None:
            all_in.append(pname)
        n_outs = len(out_names)
        donate = tuple(range(self.n_params, self.n_params + n_outs))

        def _body(*args):
            operands = list(args)
            if pname is not None:
                operands.append(partition_id_tensor())
            outs = _bass_exec_p.bind(
                *operands, out_avals=tuple(out_avals), in_names=tuple(all_in),
                out_names=tuple(out_names), lowering_input_output_aliases=(),
                sim_require_finite=False, sim_require_nnan=False, nc=nc)
            return tuple(outs)

        devices = jax.devices()[:n_cores]
        mesh = Mesh(np.asarray(devices), ("core",))
        in_specs = (PartitionSpec("core"),) * (self.n_params + n_outs)
        out_specs = (PartitionSpec("core"),) * n_outs
        self.fn = jax.jit(
            shard_map(_body, mesh=mesh, in_specs=in_specs, out_specs=out_specs,
                      check_rep=False),
            donate_argnums=donate, keep_unused=True)

    def run(self, in_maps):
        per_core = [[np.asarray(m[n]) for n in self.in_names] for m in in_maps]
        if self.dbg_name is not None:
            for pc in per_core:
                pc.append(np.zeros((1, 2), np.uint32))
        concat_in = [np.concatenate([per_core[c][i] for c in range(self.n_cores)], axis=0)
                     for i in range(self.n_params)]
        concat_zero = [np.zeros((self.n_cores * z.shape[0], *z.shape[1:]), z.dtype)
                       for z in self.zero_outs]
        out = self.fn(*concat_in, *concat_zero)
        jax.block_until_ready(out)
        return [
            {n: np.asarray(out[i]).reshape(self.n_cores, *self.out_avals[i].shape)[c]
             for i, n in enumerate(self.out_names)}
            for c in range(self.n_cores)
        ]


# ---------------------------------------------------------------- host glue

def make_inputs(cfg, inputs, per_core_o, per_core_s):
    """Build in_maps (list of per-core dicts of np arrays)."""
    NPAD, PER_CORE, NC, DDEG = cfg["NPAD"], cfg["PER_CORE"], cfg["NCORES"], cfg["DDEG"]
    N = cfg["N"]
    bf = mybir.dt.np(BF16)

    x = np.zeros((NPAD, 128), np.float32)
    x[:N] = np.asarray(inputs["x_o"], np.float32)
    xT = np.ascontiguousarray(x.T)                          # [128, NPAD]
    xTg = np.stack([xT[:, c * PER_CORE:(c + 1) * PER_CORE] for c in range(NC)]).astype(bf)

    deg = np.zeros((NPAD, DDEG), np.float32)
    deg[:N] = np.asarray(inputs["degree_o"], np.float32)

    ident = np.eye(128, dtype=np.float32).astype(bf)

    def wpack(W, a_src, a_dst):
        W = np.asarray(W, np.float64)
        return np.concatenate(
            [(W @ np.asarray(a_src, np.float64))[:, None], W,
             (W @ np.asarray(a_dst, np.float64))[:, None]], axis=1).astype(bf)

    Wp = {("o", 1): wpack(inputs["W_o1"], inputs["a_src_o1"], inputs["a_dst_o1"]),
          ("o", 2): wpack(inputs["W_o2"], inputs["a_src_o2"], inputs["a_dst_o2"]),
          ("s", 1): wpack(inputs["W_s1"], inputs["a_src_s1"], inputs["a_dst_s1"]),
          ("s", 2): wpack(inputs["W_s2"], inputs["a_src_s2"], inputs["a_dst_s2"])}
    bias = {("o", 1): inputs["b_o1"], ("o", 2): inputs["b_o2"],
            ("s", 1): inputs["b_s1"], ("s", 2): inputs["b_s2"]}
    Wpred = np.asarray(inputs["W_pred"], np.float32)
    bpred = np.asarray(inputs["b_pred"], np.float32)

    in_maps = []
    for c in range(NC):
        m = {
            "xTg": xTg,
            "x_locT": xTg[c],
            "degT": np.ascontiguousarray(deg[c * PER_CORE:(c + 1) * PER_CORE].T).astype(bf),
            "ident": ident,
            "Wpred_a": Wpred[0:128].astype(bf),
            "Wpred_b": Wpred[128:256].astype(bf),
            "Wpred_c": Wpred[256:256 + DDEG].astype(bf),
            "bpred_diff": np.full((128, 1), bpred[0] - bpred[1], np.float32),
            "idx_o": per_core_o[c]["idx"], "idx_s": per_core_s[c]["idx"],
            "ohT_o": per_core_o[c]["ohT"], "ohT_s": per_core_s[c]["ohT"],
            "oh_o": per_core_o[c]["oh"], "oh_s": per_core_s[c]["oh"],
        }
        for (g, l), w in Wp.items():
            m[f"Wp_{g}{l}"] = w
            m[f"bias_{g}{l}"] = np.tile(np.asarray(bias[g, l], np.float32)[None, :], (128, 1))
        in_maps.append(m)
    return in_maps


_CACHE = {}


def kernel(**inputs) -> np.ndarray:
    cfg = _derive(REAL_CFG)
    eo = np.asarray(inputs["edge_index_o"])
    es = np.asarray(inputs["edge_index_s"])
    per_core_o, sched_o = prep_graph(eo, cfg)
    per_core_s, sched_s = prep_graph(es, cfg)

    key = (tuple(map(tuple, sched_o)), tuple(map(tuple, sched_s)))
    if key not in _CACHE:
        nc = build_program(cfg, sched_o, sched_s)
        _CACHE[key] = SpmdRunner(nc, cfg["NCORES"])
    runner = _CACHE[key]

    in_maps = make_inputs(cfg, inputs, per_core_o, per_core_s)
    res = runner.run(in_maps)
    out = np.concatenate([res[c]["pred_out"] for c in range(cfg["NCORES"])], axis=0)
    return out[:cfg["N"]].astype(np.float32)


# revision 11
# speedup vs baseline: 120.3621x; 120.3621x over previous
"""Trainium2 Bass kernel for nn_AttnEncoder (2-branch 2-layer GAT + attn pred).

Self-contained: hardcodes problem shapes; preprocesses edges on host, compiles
one SPMD Bass program, runs on 8 NeuronCores via PJRT, returns full output.

Sharding: nodes (and their dst-sorted incoming edges) are partitioned across
8 cores; GAT weights replicated; features all-gathered between layers.

Per GAT conv (graph g, layer l):
  - node-table rows [s | h(128) | wslot | pad] (bf16, 512B rows) built on the
    PE (h' = x @ [Wa_src | W | Wa_dst]) and written to HBM; a local d-table
    [n_loc, 128] holds d broadcast per row.
  - edges (sorted by dst, grouped per 128-dst tile, bucketed by src range so
    gather indices fit int16) fetch rows edge-major via SWDGE dma_gather.
  - per-edge weights w = exp(leaky_relu(s_src + d_dst)) are computed batched
    and applied with one broadcast-AP tensor_tensor per dst tile; a streamed
    static fp8 one-hot matmul scatter-accumulates [128 dst, 128 feat + denom]
    into PSUM; epilogue divides by the denominator and adds bias (+relu L1).
Between layers: AllGather of feature-major local blocks (global x kept
block-feat-major [8, 128, n_loc] so no device transposes are needed).
"""
import os
import sys
import numpy as np

sys.path.insert(0, "/opt/trn_rl_repo")
DBG = set(os.environ.get("KERNEL_DEBUG", "").split(","))

import jax
from jax.sharding import Mesh, PartitionSpec
from jax.experimental.shard_map import shard_map

import concourse.bass as bass
import concourse.bacc as bacc
import concourse.mybir as mybir
from concourse import tile
from concourse.library_config import mlp
from concourse.bass2jax import _bass_exec_p, install_neuronx_cc_hook, partition_id_tensor

F32 = mybir.dt.float32
BF16 = mybir.dt.bfloat16
FP8 = mybir.dt.float8e4
_OH_DT = None  # set in build_program
I16 = mybir.dt.int16
AF = mybir.ActivationFunctionType
ALU = mybir.AluOpType

NEG_SLOPE = 0.2

REAL_CFG = dict(N=50000, NCORES=8, PER_CORE=6272, DDEG=32)


def _derive(cfg):
    cfg = dict(cfg)
    cfg["NPAD"] = cfg["PER_CORE"] * cfg["NCORES"]
    cfg["NT"] = cfg["PER_CORE"] // 128
    cfg["BUCKET"] = cfg["NPAD"] // 2
    assert cfg["BUCKET"] <= 32767
    return cfg


# ---------------------------------------------------------------- host prep

def _wrap16(idx: np.ndarray) -> np.ndarray:
    """int idx [n] (n%16==0) -> [128, n//16] int16 wrapped in 16 partitions,
    replicated 8x vertically (one copy per Q7 core)."""
    n = idx.shape[0]
    w = idx.reshape(n // 16, 16).T.astype(np.int16)
    return np.tile(w, (8, 1))


def prep_graph(edge_index: np.ndarray, cfg):
    """Returns (per_core list of dicts {idx, didx, oh}, sched list[(P0,P1)])."""
    NPAD, PER_CORE, NT, BUCKET, NC = (cfg["NPAD"], cfg["PER_CORE"], cfg["NT"],
                                      cfg["BUCKET"], cfg["NCORES"])
    src = edge_index[0].astype(np.int64)
    dst = edge_index[1].astype(np.int64)
    loops = np.arange(NPAD, dtype=np.int64)
    src = np.concatenate([src, loops])
    dst = np.concatenate([dst, loops])

    core = dst // PER_CORE
    dst_local = dst - core * PER_CORE
    tile_id = dst_local >> 7
    bucket = src // BUCKET

    key = (core * NT + tile_id) * 2 + bucket
    cnt = np.bincount(key, minlength=NC * NT * 2).reshape(NC, NT, 2)
    P = np.maximum.reduce(-(-cnt // 128), axis=0)          # [NT, 2]
    sched = [(int(P[t, 0]), int(P[t, 1])) for t in range(NT)]

    order = np.argsort(key, kind="stable")
    src_s, dstl_s, key_s = src[order], dst_local[order], key[order]
    seg_lo = np.searchsorted(key_s, np.arange(NC * NT * 2))
    seg_hi = np.searchsorted(key_s, np.arange(NC * NT * 2), side="right")

    fp8np = mybir.dt.np(FP8)
    per_core = []
    for c in range(NC):
        idx_parts, didx_parts, oh_parts = [], [], []
        for t in range(NT):
            d_all = []
            for b in range(2):
                p = P[t, b]
                if p == 0:
                    continue
                k = (c * NT + t) * 2 + b
                lo, hi = seg_lo[k], seg_hi[k]
                n = hi - lo
                npd = p * 128
                e_src = np.zeros(npd, dtype=np.int64)
                e_src[:n] = src_s[lo:hi] - b * BUCKET
                e_dl = np.full(npd, -1, dtype=np.int64)
                e_dl[:n] = dstl_s[lo:hi]
                idx_parts.append(_wrap16(e_src))
                d_all.append(e_dl)
            e_dl = np.concatenate(d_all)
            nch = e_dl.shape[0] // 128
            oh = np.zeros((128, nch * 128), dtype=np.float32)
            ohT = np.zeros((128, nch * 128), dtype=np.float32)
            j = np.arange(e_dl.shape[0])
            valid = e_dl >= 0
            jv = j[valid]
            col = (jv // 128) * 128 + (e_dl[valid] & 127)
            oh[jv % 128, col] = 1.0
            ohT[e_dl[valid] & 127, (jv // 128) * 128 + (jv % 128)] = 1.0
            oh_parts.append(oh)
            didx_parts.append(ohT)
        per_core.append({
            "idx": np.concatenate(idx_parts, axis=1),
            "ohT": np.concatenate(didx_parts, axis=1).astype(
                mybir.dt.np(BF16) if "nofp8" in DBG else fp8np),
            "oh": np.concatenate(oh_parts, axis=1).astype(
                mybir.dt.np(BF16) if "nofp8" in DBG else fp8np),
        })
    return per_core, sched


# ---------------------------------------------------------------- program

def build_program(cfg, sched_o, sched_s, sim_safe=False):
    NPAD, PER_CORE, NT, BUCKET, NC, DDEG = (cfg["NPAD"], cfg["PER_CORE"], cfg["NT"],
                                            cfg["BUCKET"], cfg["NCORES"], cfg["DDEG"])
    D = 128
    sched = {"o": sched_o, "s": sched_s}
    Wi = {g: sum(p0 + p1 for p0, p1 in sched[g]) * 8 for g in ("o", "s")}
    Etot = {g: sum(p0 + p1 for p0, p1 in sched[g]) * 128 for g in ("o", "s")}

    nc = bacc.Bacc("TRN2")

    xTg = nc.dram_tensor("xTg", [NC, 128, PER_CORE], BF16, kind="ExternalInput")
    x_locT = nc.dram_tensor("x_locT", [128, PER_CORE], BF16, kind="ExternalInput")
    degT_d = nc.dram_tensor("degT", [DDEG, PER_CORE], BF16, kind="ExternalInput")
    ident_d = nc.dram_tensor("ident", [128, 128], BF16, kind="ExternalInput")
    Wp_d, bias_d = {}, {}
    for g in ("o", "s"):
        for l in (1, 2):
            Wp_d[g, l] = nc.dram_tensor(f"Wp_{g}{l}", [128, 130], BF16, kind="ExternalInput")
            bias_d[g, l] = nc.dram_tensor(f"bias_{g}{l}", [128, 128], F32, kind="ExternalInput")
    Wpred_d = {k: nc.dram_tensor(f"Wpred_{k}", [128 if k != "c" else DDEG, 2], BF16,
                                 kind="ExternalInput") for k in ("a", "b", "c")}
    bpred_d = nc.dram_tensor("bpred_diff", [128, 1], F32, kind="ExternalInput")
    idx_d = {g: nc.dram_tensor(f"idx_{g}", [128, Wi[g]], I16, kind="ExternalInput")
             for g in ("o", "s")}
    OHDT = BF16 if "nofp8" in DBG else FP8
    oh_d = {g: nc.dram_tensor(f"oh_{g}", [128, Etot[g]], OHDT, kind="ExternalInput")
            for g in ("o", "s")}
    ohT_d = {g: nc.dram_tensor(f"ohT_{g}", [128, Etot[g]], OHDT, kind="ExternalInput")
             for g in ("o", "s")}

    pred_out = nc.dram_tensor("pred_out", [PER_CORE, D], F32, kind="ExternalOutput")

    table = [nc.dram_tensor(f"table{i}", [NPAD, 256], BF16) for i in range(2)]
    ag_in = {g: nc.dram_tensor(f"ag_in_{g}", [128, PER_CORE], BF16) for g in ("o", "s")}
    ag_out = {g: nc.dram_tensor(f"ag_out_{g}", [NC, 128, PER_CORE], BF16,
                                addr_space="Shared") for g in ("o", "s")}

    with tile.TileContext(nc) as tc:
        nc.gpsimd.load_library(mlp)
        with (
            tc.tile_pool(name="pers", bufs=1) as pers_p,
            tc.tile_pool(name="xblk", bufs=2) as xblk_p,
            tc.tile_pool(name="stage", bufs=4) as stage_p,
            tc.tile_pool(name="idxp", bufs=1) as idx_p,
            tc.tile_pool(name="gat", bufs=2) as gat_p,
            tc.tile_pool(name="ohp", bufs=3) as oh_p,
            tc.tile_pool(name="sc", bufs=6) as sc_p,
            tc.tile_pool(name="ep", bufs=4) as ep_p,
            tc.tile_pool(name="x1out", bufs=3) as x1out_p,
            tc.tile_pool(name="prp", bufs=4) as pr_p,
            tc.tile_pool(name="psT", bufs=2, space="PSUM") as psT_p,
            tc.tile_pool(name="psA", bufs=2, space="PSUM") as psA_p,
            tc.tile_pool(name="psR", bufs=2, space="PSUM") as psR_p,
            tc.tile_pool(name="psD", bufs=1, space="PSUM") as psD_p,
            tc.tile_pool(name="psL", bufs=1, space="PSUM") as psL_p,
        ):
            ident = pers_p.tile([128, 128], BF16, tag="ident")
            nc.sync.dma_start(out=ident[:], in_=ident_d[:])
            x_locT_sb = pers_p.tile([128, PER_CORE], BF16, tag="xlocT")
            nc.sync.dma_start(out=x_locT_sb[:], in_=x_locT[:])
            degT_sb = pers_p.tile([DDEG, PER_CORE], BF16, tag="degT")
            nc.sync.dma_start(out=degT_sb[:], in_=degT_d[:])
            bpred_sb = pers_p.tile([128, 1], F32, tag="bpred")
            nc.sync.dma_start(out=bpred_sb[:], in_=bpred_d[:])

            wp_sb, bias_sb = {}, {}
            for g in ("o", "s"):
                for l in (1, 2):
                    w_ = pers_p.tile([128, 130], BF16, tag=f"wp{g}{l}", name=f"wp{g}{l}")
                    nc.sync.dma_start(out=w_[:], in_=Wp_d[g, l][:])
                    wp_sb[g, l] = w_
                    b_ = pers_p.tile([128, 128], F32, tag=f"bb{g}{l}", name=f"bb{g}{l}")
                    nc.sync.dma_start(out=b_[:], in_=bias_d[g, l][:])
                    bias_sb[g, l] = b_
            wpred_sb = {}
            for k in ("a", "b", "c"):
                t_ = pers_p.tile([128 if k != "c" else DDEG, 2], BF16, tag=f"wpred{k}", name=f"wpred{k}")
                nc.sync.dma_start(out=t_[:], in_=Wpred_d[k][:])
                wpred_sb[k] = t_

            x1_locT = {g: pers_p.tile([128, NT * 128], BF16, tag=f"x1T{g}", name=f"x1T{g}")
                       for g in ("o", "s")}
            d_loc = [pers_p.tile([128, NT], BF16, tag=f"dloc{i}", name=f"dloc{i}") for i in range(2)]
            x2_nm = {g: pers_p.tile([128, NT * 128], BF16, tag=f"x2nm{g}", name=f"x2nm{g}")
                     for g in ("o", "s")}

            def build_table(ti, xsrc_dram, g, l, loc_lhsT):
                """Node table (h' rows) for conv (g,l) -> table[ti]; local
                d-table -> dtab[ti]. xsrc_dram: [NC, 128, PER_CORE] bf16."""
                wt = wp_sb[g, l]
                for blk in range(NC):
                    xb = xblk_p.tile([128, PER_CORE], BF16, tag="xb")
                    nc.sync.dma_start(out=xb[:], in_=xsrc_dram[blk])
                    t = 0
                    while t < NT:
                        nb = min(3, NT - t)
                        ps = psT_p.tile([128, 3 * 130], F32, tag="pT")
                        for u in range(nb):
                            nc.tensor.matmul(
                                ps[:, u * 130:(u + 1) * 130],
                                lhsT=xb[:, (t + u) * 128:(t + u + 1) * 128],
                                rhs=wt[:], start=True, stop=True)
                        st = stage_p.tile([128, 3, 256], BF16, tag="st")
                        if sim_safe:
                            nc.vector.memset(st[:, :, 129:256], 0.0)
                        if (t // 3) % 2 == 0:
                            nc.vector.tensor_copy(
                                out=st[:, 0:nb, 0:129],
                                in_=ps[:].rearrange("p (u q) -> p u q", q=130)[:, 0:nb, 0:129])
                        else:
                            nc.scalar.copy(
                                out=st[:, 0:nb, 0:129],
                                in_=ps[:].rearrange("p (u q) -> p u q", q=130)[:, 0:nb, 0:129])
                        nc.sync.dma_start(
                            out=table[ti][(blk * NT + t) * 128:(blk * NT + t + nb) * 128, :]
                                .rearrange("(u p) q -> p u q", p=128),
                            in_=st[:, 0:nb, :])
                        t += nb
                t = 0
                while t < NT:
                    nb = min(3, NT - t)
                    ps = psT_p.tile([128, 3], F32, tag="pT")
                    for u in range(nb):
                        nc.tensor.matmul(
                            ps[:, u:u + 1],
                            lhsT=loc_lhsT[:, (t + u) * 128:(t + u + 1) * 128],
                            rhs=wt[:, 129:130], start=True, stop=True)
                    nc.vector.tensor_copy(out=d_loc[ti][:, t:t + nb], in_=ps[:, 0:nb])
                    t += nb

            def conv(g, l, ti, relu, out_nm_of, out_T_of):
                """One GAT conv; out_nm_of(t)/out_T_of(t) -> dest APs [128,128]."""
                S = sched[g]
                idx_sb = idx_p.tile([128, max(Wi["o"], Wi["s"])], I16, tag="idx")
                nc.sync.dma_start(out=idx_sb[:, 0:Wi[g]], in_=idx_d[g][:])

                icol = ohcol = 0
                for t in range(NT):
                    P0, P1 = S[t]
                    PT = P0 + P1
                    gt = gat_p.tile([128, PT, 256], BF16, tag="g")
                    for b, off, Pb in ((0, 0, P0), (1, P0, P1)):
                        if Pb == 0:
                            continue
                        if "nogather" in DBG:
                            nc.sync.dma_start(
                                out=gt[:, off:off + Pb, :],
                                in_=table[ti][0:Pb * 128, :].rearrange("(c p) q -> p c q", p=128))
                        else:
                            nc.gpsimd.dma_gather(
                                out_ap=gt[:, off:off + Pb, :],
                                in_ap=table[ti][b * BUCKET:(b + 1) * BUCKET, :],
                                idxs_ap=idx_sb[:, icol:icol + Pb * 8],
                                num_idxs=Pb * 128, num_idxs_reg=Pb * 128,
                                elem_size=256, single_packet=False)
                        icol += Pb * 8
                    oht = oh_p.tile([128, PT * 128], OHDT, tag="oh")
                    nc.sync.dma_start(out=oht[:], in_=oh_d[g][:, ohcol:ohcol + PT * 128])
                    ohtT = oh_p.tile([128, PT * 128], OHDT, tag="ohT")
                    nc.sync.dma_start(out=ohtT[:], in_=ohT_d[g][:, ohcol:ohcol + PT * 128])
                    ohcol += PT * 128

                    pd = psD_p.tile([128, PT], F32, tag="pd")
                    for k in range(PT):
                        nc.tensor.matmul(
                            pd[:, k:k + 1], lhsT=ohtT[:, k * 128:(k + 1) * 128],
                            rhs=d_loc[ti][:, t:t + 1],
                            start=True, stop=True)
                    t0 = sc_p.tile([128, PT], F32, tag="t0")
                    nc.vector.tensor_tensor(out=t0[:], in0=gt[:, :, 0], in1=pd[:],
                                            op=ALU.add)
                    t1 = sc_p.tile([128, PT], F32, tag="t1")
                    nc.vector.scalar_tensor_tensor(
                        out=t1[:], in0=t0[:], scalar=NEG_SLOPE, in1=t0[:],
                        op0=ALU.mult, op1=ALU.max)
                    wcol = sc_p.tile([128, PT], BF16, tag="w")
                    nc.scalar.activation(wcol[:], t1[:], AF.Exp)
                    nc.vector.memset(gt[:, :, 129], 1.0)
                    nc.vector.tensor_tensor(
                        out=gt[:, :, 1:130], in0=gt[:, :, 1:130],
                        in1=wcol[:].unsqueeze(2).to_broadcast([128, PT, 129]),
                        op=ALU.mult)

                    pa = psA_p.tile([128, 129], F32, tag="pa")
                    for k in range(PT):
                        nc.tensor.matmul(
                            pa[:], lhsT=oht[:, k * 128:(k + 1) * 128],
                            rhs=gt[:, k, 1:130],
                            start=(k == 0), stop=(k == PT - 1))

                    rcol = ep_p.tile([128, 1], F32, tag="r")
                    nc.vector.reciprocal(rcol[:], pa[:, 128:129])
                    out_nm = out_nm_of(t)
                    if relu:
                        of = ep_p.tile([128, 128], F32, tag="of")
                        nc.vector.scalar_tensor_tensor(
                            out=of[:], in0=pa[:, 0:128], scalar=rcol[:],
                            in1=bias_sb[g, l][:], op0=ALU.mult, op1=ALU.add)
                        nc.scalar.activation(out_nm, of[:], AF.Relu)
                    else:
                        nc.vector.scalar_tensor_tensor(
                            out=out_nm, in0=pa[:, 0:128], scalar=rcol[:],
                            in1=bias_sb[g, l][:], op0=ALU.mult, op1=ALU.add)
                    if out_T_of is not None:
                        pt_ = psR_p.tile([128, 128], BF16, tag="ptr")
                        nc.tensor.transpose(out=pt_[:], in_=out_nm, identity=ident[:])
                        nc.scalar.copy(out=out_T_of(t), in_=pt_[:])

            # ---------------- layer 1 (both branches) + allgather
            def run_branch_l1(g, ti):
                build_table(ti, xTg, g, 1, x_locT_sb)

                def nm_of(t):
                    tl = x1out_p.tile([128, 128], BF16, tag="x1nm")
                    return tl[:]

                conv(g, 1, ti, relu=True, out_nm_of=nm_of,
                     out_T_of=lambda t, g=g: x1_locT[g][:, t * 128:(t + 1) * 128])
                nc.sync.dma_start(out=ag_in[g][:], in_=x1_locT[g][:])
                if "noag" not in DBG:
                    nc.gpsimd.collective_compute(
                        "AllGather", ALU.bypass,
                        replica_groups=[list(range(NC))],
                        ins=[ag_in[g][:]], outs=[ag_out[g][:]])
                else:
                    nc.sync.dma_start(out=ag_out[g][0], in_=ag_in[g][:])

            run_branch_l1("o", 0)
            run_branch_l1("s", 1)

            # ---------------- layer 2
            for gi, g in enumerate(("o", "s")):
                build_table(gi, ag_out[g], g, 2, x1_locT[g])
                conv(g, 2, gi, relu=False,
                     out_nm_of=lambda t, g=g: x2_nm[g][:, t * 128:(t + 1) * 128],
                     out_T_of=None)

            # ---------------- pred
            for t in range(NT):
                lg = psL_p.tile([128, 2], F32, tag="lg")
                first = True
                for g, wk in (("o", "a"), ("s", "b")):
                    ptr_ = psR_p.tile([128, 128], BF16, tag="ptr")
                    nc.tensor.transpose(out=ptr_[:], in_=x2_nm[g][:, t * 128:(t + 1) * 128],
                                        identity=ident[:])
                    x2T = pr_p.tile([128, 128], BF16, tag="x2T")
                    nc.scalar.copy(out=x2T[:], in_=ptr_[:])
                    nc.tensor.matmul(lg[:], lhsT=x2T[:], rhs=wpred_sb[wk][:],
                                     start=first, stop=False)
                    first = False
                nc.tensor.matmul(lg[:], lhsT=degT_sb[:, t * 128:(t + 1) * 128],
                                 rhs=wpred_sb["c"][:], start=False, stop=True)
                lgs = pr_p.tile([128, 2], F32, tag="lgs")
                nc.vector.tensor_copy(out=lgs[:], in_=lg[:])
                diff = pr_p.tile([128, 1], F32, tag="diff")
                nc.vector.tensor_tensor(out=diff[:], in0=lgs[:, 0:1], in1=lgs[:, 1:2],
                                        op=ALU.subtract)
                attn = pr_p.tile([128, 1], F32, tag="attn")
                nc.scalar.activation(attn[:], diff[:], AF.Sigmoid, bias=bpred_sb[:])
                delta = pr_p.tile([128, 128], BF16, tag="delta")
                nc.vector.tensor_tensor(
                    out=delta[:], in0=x2_nm["o"][:, t * 128:(t + 1) * 128],
                    in1=x2_nm["s"][:, t * 128:(t + 1) * 128], op=ALU.subtract)
                po = pr_p.tile([128, 128], F32, tag="po")
                nc.vector.scalar_tensor_tensor(
                    out=po[:], in0=delta[:], scalar=attn[:],
                    in1=x2_nm["s"][:, t * 128:(t + 1) * 128],
                    op0=ALU.mult, op1=ALU.add)
                nc.sync.dma_start(out=pred_out[t * 128:(t + 1) * 128, :], in_=po[:])

    nc.compile()
    return nc


# ---------------------------------------------------------------- runner

class SpmdRunner:
    def __init__(self, nc, n_cores):
        install_neuronx_cc_hook()
        self.nc = nc
        self.n_cores = n_cores
        in_names, out_names, out_avals, zero_outs = [], [], [], []
        pname = nc.partition_id_tensor.name if nc.partition_id_tensor else None
        for alloc in nc.m.functions[0].allocations:
            if not isinstance(alloc, mybir.MemoryLocationSet):
                continue
            name = alloc.memorylocations[0].name
            if alloc.kind == "ExternalInput":
                if name != pname:
                    in_names.append(name)
            elif alloc.kind == "ExternalOutput":
                out_names.append(name)
                shape = tuple(alloc.tensor_shape)
                dtype = mybir.dt.np(alloc.dtype)
                out_avals.append(jax.core.ShapedArray(shape, dtype))
                zero_outs.append(np.zeros(shape, dtype))
        self.dbg_name = nc.dbg_addr.name if nc.dbg_addr is not None else None
        self.in_names = [n for n in in_names if n != self.dbg_name]
        self.out_names, self.out_avals, self.zero_outs = out_names, out_avals, zero_outs
        all_in = list(self.in_names)
        if self.dbg_name is not None:
            all_in.append(self.dbg_name)
        self.n_params = len(all_in)
        all_in.extend(out_names)
        if pname is not None:
            all_in.append(pname)
        n_outs = len(out_names)
        donate = tuple(range(self.n_params, self.n_params + n_outs))

        def _body(*args):
            operands = list(args)
            if pname is not None:
                operands.append(partition_id_tensor())
            outs = _bass_exec_p.bind(
                *operands, out_avals=tuple(out_avals), in_names=tuple(all_in),
                out_names=tuple(out_names), lowering_input_output_aliases=(),
                sim_require_finite=False, sim_require_nnan=False, nc=nc)
            return tuple(outs)

        devices = jax.devices()[:n_cores]
        mesh = Mesh(np.asarray(devices), ("core",))
        in_specs = (PartitionSpec("core"),) * (self.n_params + n_outs)
        out_specs = (PartitionSpec("core"),) * n_outs
        self.fn = jax.jit(
            shard_map(_body, mesh=mesh, in_specs=in_specs, out_specs=out_specs,
                      check_rep=False),
            donate_argnums=donate, keep_unused=True)

    def put_inputs(self, in_maps):
        per_core = [[np.asarray(m[n]) for n in self.in_names] for m in in_maps]
        if self.dbg_name is not None:
            for pc in per_core:
                pc.append(np.zeros((1, 2), np.uint32))
        concat_in = [np.concatenate([per_core[c][i] for c in range(self.n_cores)], axis=0)
                     for i in range(self.n_params)]
        dev = [jax.device_put(a) for a in concat_in]
        jax.block_until_ready(dev)
        return dev

    def fresh_zeros(self):
        z = [jax.device_put(np.zeros((self.n_cores * t.shape[0], *t.shape[1:]), t.dtype))
             for t in self.zero_outs]
        jax.block_until_ready(z)
        return z

    def exec_only(self, dev_in, dev_zero):
        out = self.fn(*dev_in, *dev_zero)
        jax.block_until_ready(out)
        return out

    def split_out(self, out):
        return [
            {n: np.asarray(out[i]).reshape(self.n_cores, *self.out_avals[i].shape)[c]
             for i, n in enumerate(self.out_names)}
            for c in range(self.n_cores)
        ]

    def run(self, in_maps):
        dev_in = self.put_inputs(in_maps)
        return self.split_out(self.exec_only(dev_in, self.fresh_zeros()))


# ---------------------------------------------------------------- host glue

def make_inputs(cfg, inputs, per_core_o, per_core_s):
    """Build in_maps (list of per-core dicts of np arrays)."""
    NPAD, PER_CORE, NC, DDEG = cfg["NPAD"], cfg["PER_CORE"], cfg["NCORES"], cfg["DDEG"]
    N = cfg["N"]
    bf = mybir.dt.np(BF16)

    x = np.zeros((NPAD, 128), np.float32)
    x[:N] = np.asarray(inputs["x_o"], np.float32)
    xT = np.ascontiguousarray(x.T)                          # [128, NPAD]
    xTg = np.stack([xT[:, c * PER_CORE:(c + 1) * PER_CORE] for c in range(NC)]).astype(bf)

    deg = np.zeros((NPAD, DDEG), np.float32)
    deg[:N] = np.asarray(inputs["degree_o"], np.float32)

    ident = np.eye(128, dtype=np.float32).astype(bf)

    def wpack(W, a_src, a_dst):
        W = np.asarray(W, np.float64)
        return np.concatenate(
            [(W @ np.asarray(a_src, np.float64))[:, None], W,
             (W @ np.asarray(a_dst, np.float64))[:, None]], axis=1).astype(bf)

    Wp = {("o", 1): wpack(inputs["W_o1"], inputs["a_src_o1"], inputs["a_dst_o1"]),
          ("o", 2): wpack(inputs["W_o2"], inputs["a_src_o2"], inputs["a_dst_o2"]),
          ("s", 1): wpack(inputs["W_s1"], inputs["a_src_s1"], inputs["a_dst_s1"]),
          ("s", 2): wpack(inputs["W_s2"], inputs["a_src_s2"], inputs["a_dst_s2"])}
    bias = {("o", 1): inputs["b_o1"], ("o", 2): inputs["b_o2"],
            ("s", 1): inputs["b_s1"], ("s", 2): inputs["b_s2"]}
    Wpred = np.asarray(inputs["W_pred"], np.float32)
    bpred = np.asarray(inputs["b_pred"], np.float32)

    in_maps = []
    for c in range(NC):
        m = {
            "xTg": xTg,
            "x_locT": xTg[c],
            "degT": np.ascontiguousarray(deg[c * PER_CORE:(c + 1) * PER_CORE].T).astype(bf),
            "ident": ident,
            "Wpred_a": Wpred[0:128].astype(bf),
            "Wpred_b": Wpred[128:256].astype(bf),
            "Wpred_c": Wpred[256:256 + DDEG].astype(bf),
            "bpred_diff": np.full((128, 1), bpred[0] - bpred[1], np.float32),
            "idx_o": per_core_o[c]["idx"], "idx_s": per_core_s[c]["idx"],
            "ohT_o": per_core_o[c]["ohT"], "ohT_s": per_core_s[c]["ohT"],
            "oh_o": per_core_o[c]["oh"], "oh_s": per_core_s[c]["oh"],
        }
        for (g, l), w in Wp.items():
            m[f"Wp_{g}{l}"] = w
            m[f"bias_{g}{l}"] = np.tile(np.asarray(bias[g, l], np.float32)[None, :], (128, 1))
        in_maps.append(m)
    return in_maps


_CACHE = {}


def kernel(**inputs) -> np.ndarray:
    cfg = _derive(REAL_CFG)
    eo = np.asarray(inputs["edge_index_o"])
    es = np.asarray(inputs["edge_index_s"])
    per_core_o, sched_o = prep_graph(eo, cfg)
    per_core_s, sched_s = prep_graph(es, cfg)

    key = (tuple(map(tuple, sched_o)), tuple(map(tuple, sched_s)))
    if key not in _CACHE:
        nc = build_program(cfg, sched_o, sched_s)
        _CACHE[key] = SpmdRunner(nc, cfg["NCORES"])
    runner = _CACHE[key]

    in_maps = make_inputs(cfg, inputs, per_core_o, per_core_s)
    res = runner.run(in_maps)
    out = np.concatenate([res[c]["pred_out"] for c in range(cfg["NCORES"])], axis=0)
    return out[:cfg["N"]].astype(np.float32)
